# revision 37
# baseline (speedup 1.0000x reference)
"""BurstGNN Trainium2 kernel — single fused launch on 8 NeuronCores (SPMD).

Sharding: nodes/edges partitioned by dst across the 8 cores (graph
partitioning per the hint); small weights replicated; the FAConv halo
exchange is a device-side AllGather of the per-core node-table shard.

Device program (one Bass program, one launch):
  1. Encoder: props shard -> x0 (leaky-relu MLP), al/ar attention dots,
     writes the extended node-table shard [x*dinv | al].
  2. AllGather shard -> full node table (halo exchange).
  3. FAConv layer 1: per 128-dst block, indirect-DMA gather of source rows,
     alpha = tanh(al_src + ar_dst) built with a one-hot compare against the
     block-local dst offset, scatter-add via selection-matrix matmuls in
     PSUM; writes the next extended table shard.
  4. AllGather again; FAConv layer 2 + smoothing sqrt(x^2+1e-8).
  5. Per-user segment sums (users are contiguous row ranges; each core
     owns a contiguous user window) with the same one-hot machinery.
Host: builds slot metadata (numpy), applies re_index + the tiny final MLP.

Inputs are packed into three per-core blobs (f32 / bf16 / i32) to keep the
per-parameter dispatch overhead down. K_E/K_U (slot columns per 128-row
block) are derived from the input data at runtime.
"""

import os
import sys

sys.path.insert(0, "/opt/trn_rl_repo")

import ml_dtypes
import numpy as np

# Persistent XLA compile cache: makes recompiles of the identical program
# (same data -> same BIR) a cache hit across processes.
try:
    import jax
    os.makedirs("/root/jaxcache", exist_ok=True)
    jax.config.update("jax_compilation_cache_dir", "/root/jaxcache")
    jax.config.update("jax_persistent_cache_min_entry_size_bytes", -1)
    jax.config.update("jax_persistent_cache_min_compile_time_secs", 0.0)
except Exception:
    pass

import concourse.bass as bass
import concourse.bacc as bacc
import concourse.mybir as mybir
import concourse.tile as tile

F32 = mybir.dt.float32
BF16 = mybir.dt.bfloat16
I32 = mybir.dt.int32
AF = mybir.ActivationFunctionType
OP = mybir.AluOpType

# problem shapes
N, E, U = 200000, 1600000, 20000
NUMP, CATP, D = 20, 12, 64
C = 8
NS = N // C             # 25000 rows per core
NB = (NS + 127) // 128  # 196 dst blocks per core
NSP = NB * 128          # 25088 padded rows per core
TBLR = C * NSP          # 200704 global padded rows
ROWB = 66               # table row floats: x*dinv (64) | al | 1 pad
UHB = 24                # user blocks per core
UH = UHB * 128          # 3072 user slots per core window
EPS = 0.1
SLOPE = 0.01


def _v(t, dims, off=0):
    """View of a tile AP with custom free dims (keeps partition dim)."""
    return bass.AP(t.tensor, t.offset + off,
                   [list(t.ap[0])] + [list(d) for d in dims])


def _dap(handle, off, dims):
    return bass.AP(handle, int(off), [list(d) for d in dims])


class Off:
    """Element offsets inside the three packed input blobs."""

    def __init__(self, KE, KU):
        # f32 blob
        o = 0
        self.MFE = o; o += NB * 128 * KE
        self.MFU = o; o += UHB * 128 * KU
        self.DINV = o; o += 128 * NB
        self.IOTA = o; o += 128 * 128
        self.IDENT = o; o += 128 * 128
        self.WTOG = o; o += 64 * 64
        self.BNUM = o; o += 32
        self.BCAT = o; o += 32
        self.BTOG = o; o += 64
        self.BTOGR = o; o += 64
        self.ATTP = o; o += 64 * 2
        self.WF1 = o; o += 64 * 32
        self.F32SIZE = o
        # bf16 blob
        o = 0
        self.PROPS = o; o += 32 * NSP
        self.WNUM = o; o += NUMP * 32
        self.WCAT = o; o += CATP * 32
        self.BF16SIZE = o
        # i32 blob
        o = 0
        self.IDXE = o; o += NB * 128 * KE
        self.IDXU = o; o += UHB * 128 * KU
        self.I32SIZE = o


# --------------------------------------------------------------------------
# Host preprocessing
# --------------------------------------------------------------------------

def preprocess(inputs):
    src = np.asarray(inputs["edge_index"][0]).astype(np.int64, copy=False)
    dst = np.asarray(inputs["edge_index"][1]).astype(np.int64, copy=False)
    offs = np.asarray(inputs["tweet_offsets"]).astype(np.int64, copy=False)

    loop = np.arange(N, dtype=np.int64)
    srcA = np.concatenate([src, loop])
    dstA = np.concatenate([dst, loop])
    M = srcA.shape[0]

    deg = np.bincount(dstA, minlength=N).astype(np.float64)
    dinv = np.where(deg > 0, deg ** -0.5, 0.0).astype(np.float32)

    core = dstA // NS
    dloc = dstA - core * NS
    blk = dloc >> 7
    f_off = (dloc & 127).astype(np.float32)
    gblk = core * NB + blk

    cnt = np.bincount(gblk, minlength=C * NB)
    # per-block columns = max over cores so the SPMD program is uniform
    cnt2 = cnt.reshape(C, NB)
    kb_e = np.maximum(1, -(-cnt2.max(axis=0) // 128)).astype(np.int64)  # [NB]
    KE = int(kb_e.max())

    # ---- user phase ----
    seg = np.searchsorted(offs, np.arange(N, dtype=np.int64),
                          side="right") - 1
    seg = np.clip(seg, 0, U - 1)
    core_n = np.arange(N, dtype=np.int64) // NS
    ulo = seg[np.arange(C, dtype=np.int64) * NS]
    ul = seg - ulo[core_n]
    assert ul.min() >= 0 and ul.max() < UH, \
        f"user window overflow: {ul.max()}"
    ublk = ul >> 7
    uoff = (ul & 127).astype(np.float32)
    gublk = core_n * UHB + ublk
    ucnt = np.bincount(gublk, minlength=C * UHB)
    ucnt2 = ucnt.reshape(C, UHB)
    kb_u = np.maximum(1, -(-ucnt2.max(axis=0) // 128)).astype(np.int64)
    KU = int(kb_u.max())

    off = Off(KE, KU)

    iota = np.tile(np.arange(128, dtype=np.float32)[None, :], (128, 1))
    ident = np.eye(128, dtype=np.float32)
    wtog = np.asarray(inputs["W_tog"], np.float32)
    bnum = np.asarray(inputs["b_num"], np.float32)
    bcat = np.asarray(inputs["b_cat"], np.float32)
    btog = np.asarray(inputs["b_tog"], np.float32)
    attp = np.stack([np.asarray(inputs["att_l"], np.float32),
                     np.asarray(inputs["att_r"], np.float32)], axis=1)
    wf1 = np.asarray(inputs["W_f1"], np.float32)
    wnum = np.asarray(inputs["W_num"], np.float32).astype(ml_dtypes.bfloat16)
    wcat = np.asarray(inputs["W_cat"], np.float32).astype(ml_dtypes.bfloat16)
    num = np.asarray(inputs["num_prop"], np.float32)
    cat = np.asarray(inputs["cat_prop"], np.float32)

    return dict(KE=KE, KU=KU, kb_e=kb_e, kb_u=kb_u, ulo=ulo, off=off,
                gblk=gblk, cnt=cnt, srcA=srcA, f_off=f_off, M=M,
                gublk=gublk, uoff=uoff, core_n=core_n, dinv=dinv,
                iota=iota, ident=ident, wtog=wtog, bnum=bnum, bcat=bcat,
                btog=btog, attp=attp, wf1=wf1, wnum=wnum, wcat=wcat,
                num=num, cat=cat)


def pack_blobs(pre):
    """Slot-array construction + per-core blob packing (runs on a worker
    thread, overlapped with the program build)."""
    off = pre["off"]
    KE, KU = pre["KE"], pre["KU"]
    gblk, cnt, srcA, f_off, M = (pre["gblk"], pre["cnt"], pre["srcA"],
                                 pre["f_off"], pre["M"])
    gublk, uoff, core_n = pre["gublk"], pre["uoff"], pre["core_n"]
    dinv, iota, ident = pre["dinv"], pre["iota"], pre["ident"]
    wtog, bnum, bcat, btog = pre["wtog"], pre["bnum"], pre["bcat"], pre["btog"]
    attp, wf1, wnum, wcat = pre["attp"], pre["wf1"], pre["wnum"], pre["wcat"]
    num, cat = pre["num"], pre["cat"]

    order = np.argsort(gblk, kind="stable")
    starts = np.zeros(C * NB + 1, np.int64)
    np.cumsum(cnt, out=starts[1:])
    ranks = np.arange(M, dtype=np.int64) - starts[gblk[order]]
    kk = ranks >> 7
    pp = ranks & 127
    gpos = gblk[order] * (128 * KE) + pp * KE + kk
    idx_e = np.zeros(C * NB * 128 * KE, np.int32)
    mf_e = np.full(C * NB * 128 * KE, -1.0, np.float32)
    so = srcA[order]
    sc = so // NS
    idx_e[gpos] = (sc * NSP + (so - sc * NS)).astype(np.int32)
    mf_e[gpos] = f_off[order]
    idx_e = idx_e.reshape(C, NB * 128 * KE)
    mf_e = mf_e.reshape(C, NB * 128 * KE)

    change = np.empty(N, bool)
    change[0] = True
    np.not_equal(gublk[1:], gublk[:-1], out=change[1:])
    run_id = np.cumsum(change) - 1
    run_start = np.flatnonzero(change)
    ranks_u = np.arange(N, dtype=np.int64) - run_start[run_id]
    ku = ranks_u >> 7
    pu = ranks_u & 127
    gposu = gublk * (128 * KU) + pu * KU + ku
    idx_u = np.zeros(C * UHB * 128 * KU, np.int32)
    mf_u = np.full(C * UHB * 128 * KU, -1.0, np.float32)
    idx_u[gposu] = (np.arange(N, dtype=np.int64) - core_n * NS).astype(np.int32)
    mf_u[gposu] = uoff
    idx_u = idx_u.reshape(C, UHB * 128 * KU)
    mf_u = mf_u.reshape(C, UHB * 128 * KU)
    blob_f = np.empty((C, off.F32SIZE), np.float32)
    blob_h = np.zeros((C, off.BF16SIZE), ml_dtypes.bfloat16)
    blob_i = np.empty((C, off.I32SIZE), np.int32)
    for c in range(C):
        bf = blob_f[c]
        bf[off.MFE:off.MFE + mf_e.shape[1]] = mf_e[c]
        bf[off.MFU:off.MFU + mf_u.shape[1]] = mf_u[c]
        sl = slice(c * NS, (c + 1) * NS)
        dl = np.zeros(NSP, np.float32)
        dl[:NS] = dinv[sl]
        bf[off.DINV:off.DINV + 128 * NB] = dl.reshape(NB, 128).T.ravel()
        bf[off.IOTA:off.IOTA + 128 * 128] = iota.ravel()
        bf[off.IDENT:off.IDENT + 128 * 128] = ident.ravel()
        bf[off.WTOG:off.WTOG + 64 * 64] = wtog.ravel()
        bf[off.BNUM:off.BNUM + 32] = bnum
        bf[off.BCAT:off.BCAT + 32] = bcat
        bf[off.BTOG:off.BTOG + 64] = btog
        bf[off.BTOGR:off.BTOGR + 64] = btog
        bf[off.ATTP:off.ATTP + 128] = attp.ravel()
        bf[off.WF1:off.WF1 + 64 * 32] = wf1.ravel()
        bh = blob_h[c]
        pr = np.zeros((32, NSP), np.float32)
        pr[0:NUMP, 0:NS] = num[sl].T
        pr[NUMP:32, 0:NS] = cat[sl].T
        bh[off.PROPS:off.PROPS + 32 * NSP] = pr.astype(ml_dtypes.bfloat16).ravel()
        bh[off.WNUM:off.WNUM + NUMP * 32] = wnum.ravel()
        bh[off.WCAT:off.WCAT + CATP * 32] = wcat.ravel()
        bi = blob_i[c]
        bi[off.IDXE:off.IDXE + idx_e.shape[1]] = idx_e[c]
        bi[off.IDXU:off.IDXU + idx_u.shape[1]] = idx_u[c]

    return blob_f, blob_h, blob_i


# --------------------------------------------------------------------------
# Device program
# --------------------------------------------------------------------------

def _warm_build():
    """Tiny throwaway build: settles one-time builder state so the real
    program's BIR bytes are identical in every process (compile-cache key)."""
    nc = bacc.Bacc(disable_frame_to_traceback=True)
    a = nc.declare_dram_parameter("a", [128, 128], F32, isOutput=False)
    o = nc.declare_dram_parameter("o", [128, 128], F32, isOutput=True)
    with tile.TileContext(nc) as tc:
        with tc.tile_pool(name="p", bufs=1) as p:
            t = p.tile([128, 128], F32)
            nc.sync.dma_start(out=t[:], in_=a[:, :])
            nc.sync.dma_start(out=o[:, :], in_=t[:])
    nc.finalize()


_warm_build()


def _strip_debug(nc):
    """Remove caller-stack debug records from every instruction so the BIR
    bytes are identical regardless of who calls the builder (stable compile
    cache key across processes)."""
    for f in nc.m.functions:
        for b in f.blocks:
            for ins in b.instructions:
                if ins.debug is not None:
                    ins.debug = None
                try:
                    if ins.bass_addl_debug is not None:
                        ins.bass_addl_debug = None
                except AttributeError:
                    pass
        for alloc in f.allocations:
            mls = getattr(alloc, "memorylocations", None)
            if mls:
                for ml in mls:
                    if getattr(ml, "ant_debug", None) is not None:
                        try:
                            ml.ant_debug = None
                        except Exception:
                            pass


def build_program(KE, KU, kb_e, kb_u):
    kb_e = [int(x) for x in kb_e]
    kb_u = [int(x) for x in kb_u]
    off = Off(KE, KU)
    nc = bacc.Bacc(disable_frame_to_traceback=True)

    bfp = nc.declare_dram_parameter("bfp", [off.F32SIZE], F32, isOutput=False)
    bhp = nc.declare_dram_parameter("bhp", [off.BF16SIZE], BF16, isOutput=False)
    bip = nc.declare_dram_parameter("bip", [off.I32SIZE], I32, isOutput=False)
    usums = nc.declare_dram_parameter("usums", [UH, 32], BF16, isOutput=True)

    shard0 = nc.dram_tensor("shard0", [NSP, ROWB], F32, kind="Internal")
    shard1 = nc.dram_tensor("shard1", [NSP, ROWB], F32, kind="Internal")
    xg0 = nc.dram_tensor("xg0", [TBLR, ROWB], F32, kind="Internal")
    xg1 = nc.dram_tensor("xg1", [TBLR, ROWB], F32, kind="Internal")
    arr0 = nc.dram_tensor("arr0", [NSP, 1], F32, kind="Internal")
    arr1 = nc.dram_tensor("arr1", [NSP, 1], F32, kind="Internal")
    x2sloc = nc.dram_tensor("x2sloc", [NSP, 64], F32, kind="Internal")

    with tile.TileContext(nc) as tc:
        with tc.tile_pool(name="consts", bufs=1) as cp:
            wnum_s = cp.tile([NUMP, 32], BF16)
            nc.sync.dma_start(out=wnum_s[:],
                              in_=_dap(bhp, off.WNUM, [[32, NUMP], [1, 32]]))
            wcat_s = cp.tile([32 + CATP, 32], BF16)
            nc.sync.dma_start(out=wcat_s[32:32 + CATP, :],
                              in_=_dap(bhp, off.WCAT, [[32, CATP], [1, 32]]))
            wtog_s = cp.tile([64, 64], F32)
            nc.sync.dma_start(out=wtog_s[:],
                              in_=_dap(bfp, off.WTOG, [[64, 64], [1, 64]]))
            bnum_s = cp.tile([32, 1], F32)
            nc.sync.dma_start(out=bnum_s[:],
                              in_=_dap(bfp, off.BNUM, [[1, 32], [1, 1]]))
            bcat_s = cp.tile([32, 1], F32)
            nc.sync.dma_start(out=bcat_s[:],
                              in_=_dap(bfp, off.BCAT, [[1, 32], [1, 1]]))
            btog_s = cp.tile([64, 1], F32)
            nc.sync.dma_start(out=btog_s[:],
                              in_=_dap(bfp, off.BTOG, [[1, 64], [1, 1]]))
            btog_b = cp.tile([128, 64], F32)
            nc.sync.dma_start(out=btog_b[:],
                              in_=_dap(bfp, off.BTOGR, [[0, 128], [1, 64]]))
            attp_s = cp.tile([64, 2], F32)
            nc.sync.dma_start(out=attp_s[:],
                              in_=_dap(bfp, off.ATTP, [[2, 64], [1, 2]]))
            iota_s = cp.tile([128, 128], F32)
            nc.sync.dma_start(out=iota_s[:],
                              in_=_dap(bfp, off.IOTA, [[128, 128], [1, 128]]))
            ident_s = cp.tile([128, 128], F32)
            nc.sync.dma_start(out=ident_s[:],
                              in_=_dap(bfp, off.IDENT, [[128, 128], [1, 128]]))
            dinv_s = cp.tile([128, NB], F32)
            nc.sync.dma_start(out=dinv_s[:],
                              in_=_dap(bfp, off.DINV, [[NB, 128], [1, NB]]))
            wf1_s = cp.tile([64, 32], F32)
            nc.sync.dma_start(out=wf1_s[:],
                              in_=_dap(bfp, off.WF1, [[32, 64], [1, 32]]))
            eps_s = cp.tile([128, 1], F32)
            nc.vector.memset(eps_s[:], 1e-8)
            # x0 for the whole shard stays resident in SBUF: [128, NB*64]
            # (block b occupies columns b*64:(b+1)*64)
            x0all = cp.tile([128, NB * 64], F32)

            # ---------------- encoder ----------------
            with tc.tile_pool(name="enc", bufs=3) as ep, \
                 tc.tile_pool(name="encps", bufs=2, space="PSUM") as epp:
                for i in range(NB // 4):
                    r0 = i * 512
                    pT = ep.tile([44, 512], BF16, tag="pT")
                    nc.sync.dma_start(
                        out=pT[0:NUMP, :],
                        in_=_dap(bhp, off.PROPS + r0, [[NSP, NUMP], [1, 512]]))
                    nc.sync.dma_start(
                        out=pT[32:44, :],
                        in_=_dap(bhp, off.PROPS + NUMP * NSP + r0,
                                 [[NSP, CATP], [1, 512]]))
                    psH = epp.tile([64, 512], F32, tag="psH")
                    nc.tensor.matmul(out=psH[0:32, :], lhsT=wnum_s[:],
                                     rhs=pT[0:NUMP, :], start=True, stop=True)
                    nc.tensor.matmul(out=psH[32:64, :],
                                     lhsT=wcat_s[32:32 + CATP, :],
                                     rhs=pT[32:44, :],
                                     start=True, stop=True)
                    hT = ep.tile([64, 512], F32, tag="hT")
                    nc.scalar.activation(out=hT[0:32, :], in_=psH[0:32, :],
                                         func=AF.Identity, bias=bnum_s[:, 0:1])
                    nc.scalar.activation(out=hT[32:64, :], in_=psH[32:64, :],
                                         func=AF.Identity, bias=bcat_s[:, 0:1])
                    hT2 = ep.tile([64, 512], F32, tag="hT2")
                    nc.vector.scalar_tensor_tensor(
                        out=hT2[:], in0=hT[:], scalar=SLOPE, in1=hT[:],
                        op0=OP.mult, op1=OP.max)
                    # transposed x for al/ar
                    psX = epp.tile([64, 512], F32, tag="psX")
                    nc.tensor.matmul(out=psX[:], lhsT=wtog_s[:], rhs=hT2[:],
                                     start=True, stop=True)
                    xT = ep.tile([64, 512], F32, tag="xT")
                    nc.scalar.activation(out=xT[:], in_=psX[:],
                                         func=AF.Identity, bias=btog_s[:, 0:1])
                    xTl = ep.tile([64, 512], F32, tag="xTl")
                    nc.vector.scalar_tensor_tensor(
                        out=xTl[:], in0=xT[:], scalar=SLOPE, in1=xT[:],
                        op0=OP.mult, op1=OP.max)
                    for j in range(4):
                        b = i * 4 + j
                        js = slice(j * 128, (j + 1) * 128)
                        # row-major x block
                        xps = epp.tile([128, 64], F32, tag="xps")
                        nc.tensor.matmul(out=xps[:], lhsT=hT2[:, js],
                                         rhs=wtog_s[:], start=True, stop=True)
                        xb = ep.tile([128, 64], F32, tag="xb")
                        nc.vector.tensor_tensor(out=xb[:], in0=xps[:],
                                                in1=btog_b[:], op=OP.add)
                        xs = x0all[:, b * 64:(b + 1) * 64]
                        nc.vector.scalar_tensor_tensor(
                            out=xs, in0=xb[:], scalar=SLOPE, in1=xb[:],
                            op0=OP.mult, op1=OP.max)
                        # al/ar
                        aps = epp.tile([128, 2], F32, tag="aps")
                        nc.tensor.matmul(out=aps[:], lhsT=xTl[:, js],
                                         rhs=attp_s[:], start=True, stop=True)
                        asb = ep.tile([128, 2], F32, tag="asb")
                        nc.scalar.copy(out=asb[:], in_=aps[:])
                        # extended table row block
                        ext = ep.tile([128, ROWB], F32, tag="ext")
                        nc.vector.tensor_tensor(
                            out=ext[:, 0:64], in0=xs,
                            in1=_v(dinv_s[:], [[0, 64]], off=b), op=OP.mult)
                        nc.scalar.copy(out=ext[:, 64:65], in_=asb[:, 0:1])
                        nc.sync.dma_start(
                            out=shard0[b * 128:(b + 1) * 128, 0:65],
                            in_=ext[:, 0:65])
                        nc.sync.dma_start(
                            out=_dap(arr0, b * 128, [[1, 128], [1, 1]]),
                            in_=asb[:, 1:2])

            nc.gpsimd.collective_compute(
                "AllGather", OP.bypass, replica_groups=[list(range(C))],
                ins=[shard0[:, :].opt()], outs=[xg0[:, :].opt()])

            # ---------------- FAConv layers ----------------
            def fa_layer(lp, lpp, xg, arr, out_layer):
                for bg in range(NB // 4):
                    # metadata for 4 blocks in one DMA each
                    it4 = lp.tile([128, 4 * KE], I32, tag="it")
                    nc.sync.dma_start(
                        out=it4[:],
                        in_=_dap(bip, off.IDXE + bg * 4 * 128 * KE,
                                 [[KE, 128], [128 * KE, 4], [1, KE]]))
                    mt4 = lp.tile([128, 4 * KE], F32, tag="mt")
                    nc.sync.dma_start(
                        out=mt4[:],
                        in_=_dap(bfp, off.MFE + bg * 4 * 128 * KE,
                                 [[KE, 128], [128 * KE, 4], [1, KE]]))
                    artb4 = lp.tile([128, 512], F32, tag="artb")
                    nc.sync.dma_start(
                        out=artb4[:],
                        in_=_dap(arr, bg * 512, [[0, 128], [1, 512]]))
                    g4 = lp.tile([128, 4 * KE, ROWB], F32, tag="g")
                    for j in range(4):
                        for k in range(kb_e[bg * 4 + j]):
                            c = j * KE + k
                            nc.gpsimd.indirect_dma_start(
                                out=g4[:, c, :], out_offset=None, in_=xg[:, :],
                                in_offset=bass.IndirectOffsetOnAxis(
                                    ap=it4[:, c:c + 1], axis=0))
                    # alpha = tanh(al_src + ar_dst), all 4 blocks at once
                    # (pad columns hold garbage but are never consumed by
                    # the matmuls, which are bounded by kb_e[b])
                    t1 = lp.tile([128, 4 * KE, 128], F32, tag="t1")
                    nc.vector.tensor_tensor(
                        out=_v(t1[:], [[KE * 128, 4], [128, KE], [1, 128]]),
                        in0=_v(g4[:], [[KE * ROWB, 4], [ROWB, KE], [0, 128]],
                               off=64),
                        in1=_v(artb4[:], [[128, 4], [0, KE], [1, 128]]),
                        op=OP.add)
                    nc.scalar.activation(
                        out=_v(t1[:], [[1, 4 * KE * 128]]),
                        in_=_v(t1[:], [[1, 4 * KE * 128]]), func=AF.Tanh)
                    m01 = lp.tile([128, 4 * KE, 128], F32, tag="m01")
                    nc.vector.tensor_tensor(
                        out=m01[:],
                        in0=_v(mt4[:], [[1, 4 * KE], [0, 128]]),
                        in1=_v(iota_s[:], [[0, 4 * KE], [1, 128]]),
                        op=OP.is_equal)
                    nc.vector.tensor_tensor(
                        out=t1[:], in0=t1[:], in1=m01[:], op=OP.mult)
                    for j in range(4):
                        b = bg * 4 + j
                        K = kb_e[b]
                        fa_block(lp, lpp, out_layer, b, K, g4, t1, j)

            def fa_block(lp, lpp, out_layer, b, K, g4, t1, j):
                    agg = lpp.tile([128, 64], F32, tag="agg")
                    for k in range(K):
                        c = j * KE + k
                        nc.tensor.matmul(out=agg[:], lhsT=t1[:, c, :],
                                         rhs=g4[:, c, 0:64],
                                         start=(k == 0), stop=(k == K - 1))
                    x1 = lp.tile([128, 64], F32, tag="x1")
                    nc.vector.tensor_tensor(
                        out=x1[:], in0=agg[:],
                        in1=_v(dinv_s[:], [[0, 64]], off=b), op=OP.mult)
                    nc.vector.scalar_tensor_tensor(
                        out=x1[:], in0=x0all[:, b * 64:(b + 1) * 64],
                        scalar=EPS, in1=x1[:],
                        op0=OP.mult, op1=OP.add)
                    if out_layer == 1:
                        x1t_ps = lpp.tile([64, 128], F32, tag="x1t")
                        nc.tensor.transpose(out=x1t_ps[:], in_=x1[:],
                                            identity=ident_s[:])
                        x1t = lp.tile([64, 128], F32, tag="x1ts")
                        nc.scalar.copy(out=x1t[:], in_=x1t_ps[:])
                        aps = lpp.tile([128, 2], F32, tag="aps1")
                        nc.tensor.matmul(out=aps[:], lhsT=x1t[:],
                                         rhs=attp_s[:], start=True, stop=True)
                        asb = lp.tile([128, 2], F32, tag="asb1")
                        nc.scalar.copy(out=asb[:], in_=aps[:])
                        ext = lp.tile([128, ROWB], F32, tag="ext1")
                        nc.vector.tensor_tensor(
                            out=ext[:, 0:64], in0=x1[:],
                            in1=_v(dinv_s[:], [[0, 64]], off=b), op=OP.mult)
                        nc.scalar.copy(out=ext[:, 64:65], in_=asb[:, 0:1])
                        nc.sync.dma_start(
                            out=shard1[b * 128:(b + 1) * 128, 0:65],
                            in_=ext[:, 0:65])
                        nc.sync.dma_start(
                            out=_dap(arr1, b * 128, [[1, 128], [1, 1]]),
                            in_=asb[:, 1:2])
                    else:
                        nc.vector.tensor_tensor(out=x1[:], in0=x1[:],
                                                in1=x1[:], op=OP.mult)
                        x2s = lp.tile([128, 64], F32, tag="x2s")
                        nc.scalar.activation(out=x2s[:], in_=x1[:],
                                             func=AF.Sqrt,
                                             bias=eps_s[:, 0:1])
                        nc.sync.dma_start(
                            out=x2sloc[b * 128:(b + 1) * 128, :], in_=x2s[:])

            with tc.tile_pool(name="lay1", bufs=2) as lp, \
                 tc.tile_pool(name="lay1ps", bufs=2, space="PSUM") as lpp:
                fa_layer(lp, lpp, xg0, arr0, 1)

            nc.gpsimd.collective_compute(
                "AllGather", OP.bypass, replica_groups=[list(range(C))],
                ins=[shard1[:, :].opt()], outs=[xg1[:, :].opt()])

            with tc.tile_pool(name="lay2", bufs=2) as lp, \
                 tc.tile_pool(name="lay2ps", bufs=2, space="PSUM") as lpp:
                fa_layer(lp, lpp, xg1, arr1, 2)

            # ---------------- user segment sums ----------------
            with tc.tile_pool(name="usr", bufs=2) as up, \
                 tc.tile_pool(name="usrps", bufs=2, space="PSUM") as upp:
                for ug4 in range(UHB // 4):
                    it4 = up.tile([128, 4 * KU], I32, tag="uit")
                    nc.sync.dma_start(
                        out=it4[:],
                        in_=_dap(bip, off.IDXU + ug4 * 4 * 128 * KU,
                                 [[KU, 128], [128 * KU, 4], [1, KU]]))
                    mt4 = up.tile([128, 4 * KU], F32, tag="umt")
                    nc.sync.dma_start(
                        out=mt4[:],
                        in_=_dap(bfp, off.MFU + ug4 * 4 * 128 * KU,
                                 [[KU, 128], [128 * KU, 4], [1, KU]]))
                    for j in range(4):
                        ub = ug4 * 4 + j
                        K = kb_u[ub]
                        g = up.tile([128, KU, 64], F32, tag="ug")
                        for k in range(K):
                            nc.gpsimd.indirect_dma_start(
                                out=g[:, k, :], out_offset=None,
                                in_=x2sloc[:, :],
                                in_offset=bass.IndirectOffsetOnAxis(
                                    ap=it4[:, j * KU + k:j * KU + k + 1],
                                    axis=0))
                        m01 = up.tile([128, KU, 128], F32, tag="um01")
                        nc.vector.tensor_tensor(
                            out=m01[:, 0:K, :],
                            in0=_v(mt4[:], [[1, K], [0, 128]], off=j * KU),
                            in1=_v(iota_s[:], [[0, K], [1, 128]]),
                            op=OP.is_equal)
                        ups_t = upp.tile([128, 64], F32, tag="ups")
                        for k in range(K):
                            nc.tensor.matmul(out=ups_t[:], lhsT=m01[:, k, :],
                                             rhs=g[:, k, :],
                                             start=(k == 0),
                                             stop=(k == K - 1))
                        us = up.tile([128, 64], F32, tag="us")
                        nc.scalar.copy(out=us[:], in_=ups_t[:])
                        # project through W_f1 on device (linear, commutes
                        # with the cross-core boundary-user sum)
                        ust_ps = upp.tile([64, 128], F32, tag="ust")
                        nc.tensor.transpose(out=ust_ps[:], in_=us[:],
                                            identity=ident_s[:])
                        ust = up.tile([64, 128], F32, tag="usts")
                        nc.scalar.copy(out=ust[:], in_=ust_ps[:])
                        up_ps = upp.tile([128, 32], F32, tag="upj")
                        nc.tensor.matmul(out=up_ps[:], lhsT=ust[:],
                                         rhs=wf1_s[:], start=True, stop=True)
                        us32 = up.tile([128, 32], BF16, tag="us32")
                        nc.scalar.copy(out=us32[:], in_=up_ps[:])
                        nc.sync.dma_start(
                            out=usums[ub * 128:(ub + 1) * 128, :], in_=us32[:])

    nc.finalize()
    _strip_debug(nc)
    return nc


_PROG = {}


def _get_program(KE, KU, kb_e, kb_u):
    key = (KE, KU, tuple(int(x) for x in kb_e), tuple(int(x) for x in kb_u))
    if key not in _PROG:
        _PROG[key] = build_program(KE, KU, kb_e, kb_u)
    return _PROG[key]


# --------------------------------------------------------------------------
# Entry point
# --------------------------------------------------------------------------

def run_all(inputs, runner):
    import threading
    meta = preprocess(inputs)
    holder = {}

    def _pack():
        try:
            holder["blobs"] = pack_blobs(meta)
        except BaseException as e:  # re-raised on join
            holder["err"] = e

    th = threading.Thread(target=_pack)
    th.start()
    nc = _get_program(meta["KE"], meta["KU"], meta["kb_e"], meta["kb_u"])
    th.join()
    if "err" in holder:
        raise holder["err"]
    blob_f, blob_h, blob_i = holder["blobs"]

    maps = [{"bfp": blob_f[c], "bhp": blob_h[c],
             "bip": blob_i[c]} for c in range(C)]
    res = runner(nc, maps)

    totals = np.zeros((U, 32), np.float32)
    ulo = meta["ulo"]
    for c in range(C):
        lo = int(ulo[c])
        hi = min(lo + UH, U)
        totals[lo:hi] += res[c]["usums"][:hi - lo].astype(np.float32)

    re_index = np.asarray(inputs["re_index"]).astype(np.int64, copy=False)
    x3p = totals[re_index]
    bf1 = np.asarray(inputs["b_f1"], np.float32)
    wlab = np.asarray(inputs["W_lab"], np.float32)
    blab = np.asarray(inputs["b_lab"], np.float32)
    h = x3p + bf1
    h = np.where(h > 0, h, np.float32(SLOPE) * h)
    return (h @ wlab + blab).astype(np.float32)


def kernel(**inputs):
    from concourse.bass_utils import run_bass_kernel_spmd

    def runner(nc, in_maps):
        return run_bass_kernel_spmd(nc, in_maps,
                                    core_ids=list(range(C))).results

    return run_all(inputs, runner)
